# revision 21
# baseline (speedup 1.0000x reference)
"""Trainium2 Bass kernel for the ExtractModel edit-distance retrieval problem.

Computes, for every (batch b, start position l, span width w, vocab entry v):
    nll[b,l,w,v] = edit_distance(seq window at (b,l) of width w+1, vocab[v])
with substitution cost sub[lost, known] = -log_softmax(unit_aligner) and
insertion/deletion cost d = ins_del_scale * 3.5, plus best-span outputs.

Strategy (8 NeuronCores, SPMD, no collectives):
  - Shard the vocab axis V: global stable sort by word length, dealt
    round-robin so every shard has identical per-length bucket sizes
    (padded with dummy entries whose outputs are discarded on host).
  - Host precomputes C'[v, b, p, i] = sub[seq[b, min(p, L-1)], vocab[v, i]]
    for p in [0, L+W-1) (the window clamp baked in), so the device does
    zero gather work.
  - Device layout: vocab entries on partitions (2 chunks of 128 per core),
    (b, l, j) on the free dim (j = window slot, 11 per (b,l) group).
    DP row i (vocab char i):
        A[j]   = f_prev[j-1] + C'[b, l+j-1, i]      (GPSIMD tensor adds)
        m[j]   = min(f_prev[j] + d, A[j])           (DVE scalar_tensor_tensor)
        f[j]   = min(f[j-1] + d0[j], m[j])          (DVE tensor_tensor_scan)
    where d0 = +1e30 at each group's j==0 slot (state reset) and d elsewhere.
    All f32, bit-exact w.r.t. the reference scan's IEEE arithmetic.
  - When the length-i bucket finalizes, its partitions' DP rows are DMA'd
    straight to DRAM; the host reassembles nll and computes the (exact)
    min/argmin/score/argmax epilogue in numpy.
"""

import os
import sys

import numpy as np

# ---------------------------------------------------------------- constants
B, L = 4, 128
V, M = 2000, 10
W = 10
LU, KU = 33, 29
MIN_LEN = 4
INS_DEL = 3.5
NCORES = 8

NJ = W + 1          # window slots per (b,l) group (incl. j=0 reset slot)
PW = L + W - 1      # clamped position axis width (137)
GFD = B * L * NJ    # free-dim size of DP tiles (5632)
CFD = B * PW * M    # free-dim size of C' tiles (5480)
BIG = np.float32(1.0e30)

_REPO_PATHS = ["/opt/trn_rl_repo", "/opt/trn_rl_repo/concourse"]

_program_cache: dict = {}
_sub_cache: dict = {}


def _ensure_paths():
    for p in _REPO_PATHS:
        if os.path.isdir(p) and p not in sys.path:
            sys.path.insert(0, p)


# ---------------------------------------------------------------- host math
def _compute_sub(unit_aligner: np.ndarray) -> np.ndarray:
    """sub = -log_softmax(unit_aligner, axis=-1), matching the reference's
    jax computation (default device) as closely as possible."""
    key = unit_aligner.tobytes()
    if key in _sub_cache:
        return _sub_cache[key]
    try:
        import jax

        x = np.asarray(unit_aligner, np.float32)
        sub = np.asarray(-jax.nn.log_softmax(x, axis=-1), dtype=np.float32)
    except Exception:
        x = unit_aligner.astype(np.float32)
        mx = x.max(axis=-1, keepdims=True)
        sh = (x - mx).astype(np.float32)
        lse = np.log(np.sum(np.exp(sh), axis=-1, keepdims=True).astype(np.float32))
        sub = -(sh - lse.astype(np.float32)).astype(np.float32)
    _sub_cache[key] = sub
    return sub


def _shard_vocab(vlen: np.ndarray):
    """Stable sort by length, deal round-robin into NCORES shards, pad all
    shards to identical per-length bucket sizes; pad shard size up to a
    multiple of 128 by extending the longest bucket.

    Returns (counts[M], Vp, shard_idx[NCORES, Vp]) with shard_idx holding
    original vocab indices (-1 for dummy padding)."""
    order = np.argsort(vlen, kind="stable")
    shards = [order[k::NCORES] for k in range(NCORES)]
    counts = np.zeros(M, dtype=np.int64)
    for k in range(NCORES):
        lk = vlen[shards[k]]
        for i in range(1, M + 1):
            counts[i - 1] = max(counts[i - 1], int(np.sum(lk == i)))
    Vp = int(counts.sum())
    VpP = ((Vp + 127) // 128) * 128
    max_len = max(i for i in range(1, M + 1) if counts[i - 1] > 0)
    counts[max_len - 1] += VpP - Vp
    Vp = VpP
    shard_idx = np.full((NCORES, Vp), -1, dtype=np.int64)
    for k in range(NCORES):
        sh = shards[k]
        lk = vlen[sh]
        o = 0
        for i in range(1, M + 1):
            members = sh[lk == i]
            shard_idx[k, o : o + len(members)] = members
            o += int(counts[i - 1])
    return counts, Vp, shard_idx


def _host_build_inputs(sub, seqs, vids, d, counts, Vp, shard_idx):
    """Per-core input maps: cp_in [NCH,128,CFD], f0_in/d0_in [128,GFD]."""
    nch = Vp // 128
    jd = (np.arange(NJ, dtype=np.float32) * np.float32(d)).astype(np.float32)
    f0 = np.broadcast_to(np.tile(jd, B * L), (128, GFD)).copy()
    d0_row = np.tile(
        np.concatenate([[BIG], np.full(W, np.float32(d), np.float32)]).astype(
            np.float32
        ),
        B * L,
    )
    d0 = np.broadcast_to(d0_row, (128, GFD)).copy()

    # subs[b, p, ku] over the clamped position axis
    pos = np.minimum(np.arange(PW), L - 1)
    subs = sub[seqs[:, pos]]                     # [B, PW, KU]

    in_maps = []
    for k in range(NCORES):
        idx = shard_idx[k]
        vk = np.zeros((Vp, M), dtype=np.int64)
        real = idx >= 0
        vk[real] = vids[idx[real]]
        # cw[v, b, p, i] = subs[b, p, vk[v, i]]
        cw = np.transpose(subs[:, :, vk], (2, 0, 1, 3))  # [Vp, B, PW, M]
        cp = np.ascontiguousarray(cw, dtype=np.float32).reshape(nch, 128, CFD)
        in_maps.append({"cp_in": cp, "f0_in": f0, "d0_in": d0})
    return in_maps


# ---------------------------------------------------------------- device program
CFG: dict = {}


def _build_program(Vp: int, counts: tuple, d: float):
    """Build + compile the SPMD Bass/Tile program for the given bucket
    structure (cached)."""
    key = (Vp, counts, float(d), tuple(sorted(CFG.items())))
    if key in _program_cache:
        return _program_cache[key]

    _ensure_paths()
    import concourse.bacc as bacc
    import concourse.mybir as mybir
    import concourse.tile as tile
    from contextlib import ExitStack

    f32 = mybir.dt.float32
    nch = Vp // 128

    def o_of(i):  # global sorted position of first entry with length >= i
        return int(sum(counts[: i - 1]))

    nc = bacc.Bacc(
        "TRN2",
        target_bir_lowering=False,
        debug=False,
        enable_asserts=False,
        num_devices=NCORES,
    )

    cp_in = nc.dram_tensor("cp_in", [nch, 128, CFD], f32, kind="ExternalInput")
    f0_in = nc.dram_tensor("f0_in", [128, GFD], f32, kind="ExternalInput")
    d0_in = nc.dram_tensor("d0_in", [128, GFD], f32, kind="ExternalInput")
    nll_out = nc.dram_tensor("nll_out", [Vp, GFD], f32, kind="ExternalOutput")

    NS = int(CFG.get("ns", 4))   # independent (b,l) streams for engine overlap
    SB = B // NS                 # batches per stream
    SFD = GFD // NS              # free-dim elements per stream
    LD = int(CFG.get("ld", 0))   # l-split of fused add: [0,LD) DVE, rest GPSIMD
    STT_GP = bool(CFG.get("stt_gp", False))
    G0 = int(CFG.get("g0", 0))   # scan groups given to GPSIMD per stream

    with tile.TileContext(nc) as tc, ExitStack() as ctx:
        const_pool = ctx.enter_context(tc.tile_pool(name="const", bufs=1))
        cpool = ctx.enter_context(
            tc.tile_pool(name="cp", bufs=int(CFG.get("cpb", 2)))
        )
        fpool = ctx.enter_context(
            tc.tile_pool(name="f", bufs=int(CFG.get("fb", 3)))
        )
        mpool = ctx.enter_context(tc.tile_pool(name="m", bufs=1))
        apool = ctx.enter_context(tc.tile_pool(name="a", bufs=1))

        D0 = const_pool.tile([128, GFD], f32, tag="D0")
        nc.sync.dma_start(D0[:, :], d0_in[:, :])
        F0C = const_pool.tile([128, GFD], f32, tag="F0C")
        nc.sync.dma_start(F0C[:, :], f0_in[:, :])

        for ci in range(nch):
            lo, hi = ci * 128, (ci + 1) * 128
            # longest word whose bucket intersects this partition chunk
            max_len = max(
                i
                for i in range(1, M + 1)
                if counts[i - 1] > 0 and o_of(i) < hi and o_of(i + 1) > lo
            )

            CP = cpool.tile([128, CFD], f32)
            nc.sync.dma_start(CP[:, :], cp_in[ci, :, :])
            C4 = CP.rearrange("v (b p i) -> v b p i", p=PW, i=M)

            AFD = SB * L * W
            f_prev, At = [], []
            for s in range(NS):
                f_prev.append(F0C[:, s * SFD : (s + 1) * SFD])
                At.append(apool.tile([128, AFD], f32, tag=f"A{s}", name=f"A{s}"))

            for i in range(1, max_len + 1):
                for s in range(NS):
                    A4 = At[s].rearrange("v (b l j) -> v b l j", l=L, j=W)
                    fp4 = f_prev[s].rearrange("v (b l j) -> v b l j", l=L, j=NJ)
                    # One fused add for all (l, j): in1 reads C' with an
                    # overlapping AP where both l and j stride the position
                    # axis (p = l + j - 1).
                    cbase = C4[:, s * SB : (s + 1) * SB, 0:L, i - 1 : i]
                    dims = cbase.ap
                    dims[3] = [M, W]
                    cbase.ap = dims
                    for eng, l0, l1 in (
                        (nc.vector, 0, LD),
                        (nc.gpsimd, LD, L),
                    ):
                        if CFG.get("skip_adds") or l0 >= l1:
                            continue
                        eng.tensor_add(
                            A4[:, :, l0:l1, 0:W],
                            fp4[:, :, l0:l1, 0:W],
                            cbase[:, :, l0:l1, :],
                        )

                    mt = mpool.tile([128, SFD], f32, tag=f"m{s}")
                    m3 = mt.rearrange("v (g j) -> v g j", j=NJ)
                    fp3 = f_prev[s].rearrange("v (g j) -> v g j", j=NJ)
                    a3 = At[s].rearrange("v (g j) -> v g j", j=W)
                    # m[j0] = fp[0] + d on the otherwise-idle ACT engine
                    nc.scalar.add(m3[:, :, 0], fp3[:, :, 0], D0[:, 1:2])
                    stt_eng = nc.gpsimd if STT_GP else nc.vector
                    if not CFG.get("skip_stt"):
                        stt_eng.scalar_tensor_tensor(
                            m3[:, :, 1:NJ],
                            fp3[:, :, 1:NJ],
                            float(d),
                            a3[:, :, 0:W],
                            op0=mybir.AluOpType.add,
                            op1=mybir.AluOpType.min,
                        )

                    f_new = fpool.tile([128, SFD], f32, tag=f"f{s}")
                    scan_splits = (
                        [(nc.gpsimd, 0, G0 * NJ), (nc.vector, G0 * NJ, SFD)]
                        if G0 > 0
                        else [(nc.vector, 0, SFD)]
                    )
                    for seng, e0, e1 in scan_splits:
                        if CFG.get("skip_scan"):
                            break
                        seng.tensor_tensor_scan(
                            f_new[:, e0:e1],
                            D0[:, s * SFD + e0 : s * SFD + e1],
                            mt[:, e0:e1],
                            0.0,
                            op0=mybir.AluOpType.add,
                            op1=mybir.AluOpType.min,
                        )

                    # finalize bucket i: DMA its partitions' rows out
                    a = max(o_of(i), lo)
                    b = min(o_of(i + 1), hi)
                    if counts[i - 1] > 0 and a < b:
                        nc.scalar.dma_start(
                            nll_out[a:b, s * SFD : (s + 1) * SFD],
                            f_new[a - lo : b - lo, :],
                        )
                    f_prev[s] = f_new

    nc.compile()
    _program_cache[key] = nc
    return nc


# ---------------------------------------------------------------- entry point
def kernel(
    unit_aligner: np.ndarray,
    ins_del_scale: np.ndarray,
    unit_id_seqs: np.ndarray,
    vocab_unit_ids: np.ndarray,
    vocab_lengths: np.ndarray,
):
    _ensure_paths()

    unit_aligner = np.asarray(unit_aligner, dtype=np.float32)
    scale = np.float32(np.asarray(ins_del_scale).reshape(-1)[0])
    seqs = np.asarray(unit_id_seqs).astype(np.int64)
    vids = np.asarray(vocab_unit_ids).astype(np.int64)
    vlen = np.clip(np.asarray(vocab_lengths).astype(np.int64), 1, M)

    d = np.float32(scale * np.float32(INS_DEL))
    sub = _compute_sub(unit_aligner)

    counts, Vp, shard_idx = _shard_vocab(vlen)
    in_maps = _host_build_inputs(sub, seqs, vids, d, counts, Vp, shard_idx)

    nc = _build_program(Vp, tuple(int(x) for x in counts), float(d))

    from concourse.bass_utils import run_bass_kernel_spmd

    res = run_bass_kernel_spmd(nc, in_maps, core_ids=list(range(NCORES)))

    # ---------------- host reassembly ----------------
    nll = np.empty((B, L, W, V), dtype=np.float32)
    for k in range(NCORES):
        out = res.results[k]["nll_out"].reshape(Vp, B, L, NJ)
        idx = shard_idx[k]
        real = np.where(idx >= 0)[0]
        vorig = idx[real]
        # nll[b, l, w, vorig] = out[vs, b, l, w+1]
        nll[:, :, :, vorig] = np.transpose(out[real, :, :, 1:], (1, 2, 3, 0))

    # ---------------- best span / vocab (exact numpy replication) --------
    best_nll_v = nll.min(axis=-1)
    best_v = nll.argmin(axis=-1)
    span_len = np.arange(1, W + 1, dtype=np.float32)
    viable = (
        (np.arange(L)[:, None] + np.arange(1, W + 1)[None, :] <= L)
        & (np.arange(1, W + 1) >= MIN_LEN)[None, :]
    )
    score = np.where(
        viable[None],
        (span_len[None, None, :] - best_nll_v).astype(np.float32),
        np.float32(-np.inf),
    ).astype(np.float32)
    best_j = score.argmax(axis=-1)
    best_matched_score = score.max(axis=-1).astype(np.float32)
    best_matched_vocab = np.take_along_axis(best_v, best_j[..., None], axis=-1)[
        ..., 0
    ].astype(np.int32)

    return nll, best_matched_score, best_matched_vocab
